# revision 1
# baseline (speedup 1.0000x reference)
"""Trainium2 Bass kernel for the Attention2 module.

Computation (per batch row b):
    att_h  = h[b] @ W_h.T + b_h                      # [A]
    dot    = tanh(p_att_feats[b] + att_h)            # [L, A]
    scores = dot @ W_a[0]  (+ b_a, dropped: softmax shift-invariant)
    scores = where(mask, -1e8, scores)
    w      = softmax(scores)                         # [L]
    out[b] = w @ att_feats[b]                        # [R]

Sharding: data-parallel over batch B=32 across 8 cores (4 rows/core).

Per-core mapping (L=2048 -> 16 chunks of 128 partitions):
  phase 0: att_h for the 4 local rows via PE (K=RNN on partitions),
           +b_h on DVE, partition-broadcast via K=1 ones-matmuls.
  phase A: p-tile [128(l), 512(a)]; DVE add of broadcast att_h; ACT tanh
           (in place); DVE multiply by broadcast W_a + free-dim
           reduce_sum -> scores column [128, 1].
  softmax: no max subtraction (|scores| <= ~23 so exp can't overflow);
           ACT exp, mask applied multiplicatively (keep in {0,1}) on
           DVE, row-sum on DVE; partition sum via a ones-vector PE
           matmul; reciprocal on DVE.  exp(-1e8) == 0 in the
           reference, identical to multiplying exp(s) by 0.
  phase B: out[b] = sum_l w[l] * att_feats[b,l,:] as PE matmuls:
           lhsT = w column [128(l), 1], rhs = f-tile [128(l), 512(r)],
           accumulated over the 16 l-chunks into PSUM [1, 512] x 2.
           float32r (full-rate fp32 matmul mode) on both operands.
  scale:   result * (1/Z) on DVE, DMA out.

Host-side prep is layout only: transposes of h/W_h (so the contraction
dim lands on partitions with unit-stride loads) and the boolean mask
converted to a float keep-mask in score layout.
"""

import sys

import ml_dtypes
import numpy as np

sys.path.insert(0, "/opt/trn_rl_repo")

import concourse.bass as bass  # noqa: E402
import concourse.tile as tile  # noqa: E402
from concourse import bacc, mybir  # noqa: E402
from concourse.bass_utils import run_bass_kernel_spmd  # noqa: E402

N_CORES = 8
B, L, RNN, A = 32, 2048, 1024, 512
BS = B // N_CORES

F32 = mybir.dt.float32
F32R = mybir.dt.float32r
BF16 = mybir.dt.bfloat16
MULT = mybir.AluOpType.mult
ADD = mybir.AluOpType.add
TANH = mybir.ActivationFunctionType.Tanh
EXP = mybir.ActivationFunctionType.Exp


KERNEL_VERSION = 8

import os  # noqa: E402

_FAST = os.environ.get("KERNEL_FAST", "0")


def build_program(bs=BS, ll=L, rnn=RNN, a=A, lgrp=4, use_f32r=True,
                  p_bufs=3, f_bufs=4, add_on_gpsimd=_FAST == "1",
                  reduce_on_act=_FAST in ("1", "2")):
    nch = ll // 128          # l-chunks of 128 partitions
    ng = nch // lgrp         # DMA groups (lgrp l-chunks per transfer)
    kch = rnn // 128         # contraction chunks for att_h
    nh = (rnn + 511) // 512  # 512-wide output halves of phase B
    rh = rnn // nh           # free width per output half

    fdt = F32R if use_f32r else F32
    nc = bacc.Bacc(None, target_bir_lowering=False)
    p = nc.dram_tensor("p", [bs, ll, a], BF16, kind="ExternalInput")
    f = nc.dram_tensor("f", [bs, ll, rnn], fdt, kind="ExternalInput")
    hT = nc.dram_tensor("hT", [rnn, bs], F32, kind="ExternalInput")
    whT = nc.dram_tensor("whT", [rnn, a], F32, kind="ExternalInput")
    bh = nc.dram_tensor("bh", [1, a], F32, kind="ExternalInput")
    wa = nc.dram_tensor("wa", [1, a], F32, kind="ExternalInput")
    keep = nc.dram_tensor("keep", [bs, 128, nch], F32, kind="ExternalInput")
    # unused input whose SHAPE encodes the kernel version: the compile
    # cache keys on the HLO signature (names/shapes), NOT the embedded
    # BIR — without this, a rebuilt kernel with unchanged I/O silently
    # re-runs the previously cached NEFF.
    _code = (1 if add_on_gpsimd else 0) + (2 if reduce_on_act else 0)
    vcode = KERNEL_VERSION if _code == 0 else KERNEL_VERSION * 4 + _code
    ver = nc.dram_tensor("ver", [vcode, 1], F32, kind="ExternalInput")
    out = nc.dram_tensor("out", [bs, rnn], F32, kind="ExternalOutput")

    pr = p[:, :, :].rearrange("b (n q) a -> b q n a", q=128)
    fr = f[:, :, :].rearrange("b (n q) r -> b q n r", q=128)
    hTr = hT[:, :].rearrange("(c q) b -> q c b", q=128)
    whTr = whT[:, :].rearrange("(c q) a -> q c a", q=128)
    keepr = keep[:, :, :].rearrange("b q n -> q b n")

    with tile.TileContext(nc) as tc:
        with (
            tc.tile_pool(name="singles", bufs=1) as singles,
            tc.tile_pool(name="ppool", bufs=p_bufs) as ppool,
            tc.tile_pool(name="fpool", bufs=f_bufs) as fpool,
            tc.tile_pool(name="sm", bufs=3) as smpool,
            tc.tile_pool(name="respool", bufs=2) as respool,
            tc.tile_pool(name="psacc", bufs=2, space="PSUM") as psacc,
            tc.tile_pool(name="pssmall", bufs=2, space="PSUM") as pssmall,
        ):
            # ---- constants ----
            hT_sb = singles.tile([128, kch, bs], F32)
            nc.sync.dma_start(out=hT_sb, in_=hTr)
            whT_sb = singles.tile([128, kch, a], F32)
            nc.sync.dma_start(out=whT_sb, in_=whTr)
            # plain single-row loads (partition 0)
            bh_row = singles.tile([1, a], F32)
            nc.sync.dma_start(out=bh_row, in_=bh[:, :])
            wa_row = singles.tile([1, a], F32)
            nc.sync.dma_start(out=wa_row, in_=wa[:, :])
            keep_sb = singles.tile([128, bs, nch], F32)
            nc.sync.dma_start(out=keep_sb, in_=keepr)
            ver_sb = singles.tile([vcode, 1], F32)
            nc.sync.dma_start(out=ver_sb, in_=ver[:, :])
            ones_sb = singles.tile([128, 1], F32)
            nc.vector.memset(ones_sb, 1.0)
            # ones row for K=1 partition-broadcast matmuls
            ones_row = singles.tile([1, 128], F32)
            nc.vector.memset(ones_row, 1.0)
            wa_sb = singles.tile([128, a], BF16)
            attb_bc = singles.tile([128, bs, a], BF16)

            # ---- phase 0: attb[b] = h[b] @ W_h.T + b_h, then broadcast
            # across all 128 partitions via a K=1 ones-matmul (avoids
            # irregular 0-stride broadcast DMAs entirely).
            with tc.tile_pool(name="ps0", bufs=1, space="PSUM") as ps0:
                wa_ps = ps0.tile([128, a], F32, tag="bc")
                nc.tensor.matmul(wa_ps, lhsT=ones_row, rhs=wa_row,
                                 start=True, stop=True)
                nc.scalar.copy(out=wa_sb, in_=wa_ps)
                for b in range(bs):
                    ah_ps = ps0.tile([1, a], F32, tag="ah", name=f"ah{b}")
                    for c in range(kch):
                        nc.tensor.matmul(ah_ps, lhsT=hT_sb[:, c, b:b + 1],
                                         rhs=whT_sb[:, c, :],
                                         start=(c == 0), stop=(c == kch - 1))
                    attb_row = smpool.tile([1, a], F32, tag="attbrow")
                    nc.vector.tensor_add(attb_row, ah_ps, bh_row)
                    bc_ps = ps0.tile([128, a], F32, tag="bc", name=f"bc{b}")
                    nc.tensor.matmul(bc_ps, lhsT=ones_row, rhs=attb_row,
                                     start=True, stop=True)
                    nc.scalar.copy(out=attb_bc[:, b, :], in_=bc_ps)

            for b in range(bs):
                # ---- phase A: scores[l] = W_a . tanh(p[l] + attb) ----
                scores = smpool.tile([128, nch], F32, tag="scores")
                for t in range(ng):
                    ptile = ppool.tile([128, lgrp, a], BF16, tag="p")
                    nc.sync.dma_start(
                        out=ptile, in_=pr[b, :, t * lgrp:(t + 1) * lgrp, :])
                    add_eng = nc.gpsimd if add_on_gpsimd else nc.vector
                    for j in range(lgrp):
                        add_eng.tensor_add(
                            ptile[:, j, :], ptile[:, j, :], attb_bc[:, b, :])
                    nc.scalar.activation(out=ptile, in_=ptile, func=TANH)
                    for j in range(lgrp):
                        i = t * lgrp + j
                        # multiply by W_a on DVE, free-dim sum via the
                        # reduce primitive (tensor_tensor_reduce crashes
                        # execution on this runtime)
                        nc.vector.tensor_mul(
                            ptile[:, j, :], ptile[:, j, :], wa_sb)
                        if reduce_on_act:
                            nc.scalar.activation(
                                out=ptile[:, j, :], in_=ptile[:, j, :],
                                func=mybir.ActivationFunctionType.Copy,
                                accum_out=scores[:, i:i + 1])
                        else:
                            nc.vector.reduce_sum(
                                scores[:, i:i + 1], ptile[:, j, :],
                                axis=mybir.AxisListType.X)

                # ---- softmax weights (no max subtraction needed) ----
                e_sb = smpool.tile([128, nch], F32, tag="e")
                nc.scalar.activation(out=e_sb, in_=scores, func=EXP)
                w_sb = smpool.tile([128, nch], fdt, tag="w")
                nc.vector.tensor_mul(w_sb, e_sb, keep_sb[:, b, :])
                zpart = smpool.tile([128, 1], F32, tag="zpart")
                nc.vector.reduce_sum(zpart, w_sb, axis=mybir.AxisListType.X)
                z_ps = pssmall.tile([1, 1], F32, tag="zps")
                nc.tensor.matmul(z_ps, lhsT=ones_sb, rhs=zpart,
                                 start=True, stop=True)
                zinv = smpool.tile([1, 1], F32, tag="zinv")
                nc.vector.reciprocal(zinv, z_ps)

                # ---- phase B: out[b] = (w/Z) @ att_feats[b] ----
                rps = [psacc.tile([1, rh], F32, tag=f"r{hh}", name=f"rps{hh}")
                       for hh in range(nh)]
                for t in range(ng):
                    ftile = fpool.tile([128, lgrp, rnn], fdt, tag="f")
                    nc.sync.dma_start(
                        out=ftile, in_=fr[b, :, t * lgrp:(t + 1) * lgrp, :])
                    for j in range(lgrp):
                        i = t * lgrp + j
                        lhs = w_sb[:, i:i + 1]
                        for hh in range(nh):
                            nc.tensor.matmul(
                                rps[hh], lhsT=lhs,
                                rhs=ftile[:, j, hh * rh:(hh + 1) * rh],
                                start=(i == 0), stop=(i == nch - 1))
                res = respool.tile([1, rnn], F32, tag="res")
                for hh in range(nh):
                    nc.vector.tensor_scalar_mul(
                        res[:, hh * rh:(hh + 1) * rh], rps[hh], zinv)
                nc.sync.dma_start(out=out[b:b + 1, :], in_=res)
    nc.finalize()
    return nc


_PROG = None


def _get_program():
    global _PROG
    if _PROG is None:
        _PROG = build_program()
    return _PROG


def make_in_maps(h, att_feats, p_att_feats, mask, W_h, b_h, W_a):
    h = np.ascontiguousarray(np.asarray(h, dtype=np.float32))
    att_feats = np.asarray(att_feats, dtype=np.float32)
    p_att_feats = np.asarray(p_att_feats, dtype=np.float32)
    mask = np.asarray(mask)

    hT = np.ascontiguousarray(h.T)                                 # [RNN, B]
    whT = np.ascontiguousarray(np.asarray(W_h, np.float32).T)      # [RNN, A]
    bh = np.ascontiguousarray(np.asarray(b_h, np.float32).reshape(1, A))
    wa = np.ascontiguousarray(np.asarray(W_a, np.float32).reshape(1, A))
    # keep[b, p, i] = 1 - mask[b, i*128 + p]  (score-layout keep mask)
    keep = np.ascontiguousarray(
        (~mask).astype(np.float32).reshape(B, L // 128, 128).transpose(0, 2, 1))

    ver = np.zeros((KERNEL_VERSION, 1), np.float32)
    in_maps = []
    for c in range(N_CORES):
        s = slice(c * BS, (c + 1) * BS)
        in_maps.append({
            "p": np.ascontiguousarray(
                p_att_feats[s].astype(ml_dtypes.bfloat16)),
            "f": np.ascontiguousarray(att_feats[s]),
            "hT": np.ascontiguousarray(hT[:, s]),
            "whT": whT,
            "bh": bh,
            "wa": wa,
            "keep": np.ascontiguousarray(keep[s]),
            "ver": ver,
        })
    return in_maps


def run_sharded(inputs, trace=False, **kwargs):
    nc = _get_program()
    in_maps = make_in_maps(
        inputs["h"], inputs["att_feats"], inputs["p_att_feats"],
        inputs["mask"], inputs["W_h"], inputs["b_h"], inputs["W_a"])
    return run_bass_kernel_spmd(nc, in_maps, core_ids=list(range(N_CORES)),
                                trace=trace, **kwargs)


def kernel(h, att_feats, p_att_feats, mask, W_h, b_h, W_a, b_a):
    res = run_sharded({
        "h": h, "att_feats": att_feats, "p_att_feats": p_att_feats,
        "mask": mask, "W_h": W_h, "b_h": b_h, "W_a": W_a, "b_a": b_a})
    return np.concatenate([res.results[c]["out"] for c in range(N_CORES)],
                          axis=0).astype(np.float32)



# revision 11
# speedup vs baseline: 1.4998x; 1.4998x over previous
"""Trainium2 Bass kernel for the Attention2 module (v2: sparse-compacted).

Computation (per batch row b):
    att_h  = h[b] @ W_h.T + b_h                      # [A]
    dot    = tanh(p_att_feats[b] + att_h)            # [L, A]
    scores = dot @ W_a[0]  (+ b_a, dropped: softmax shift-invariant)
    scores = where(mask, -1e8, scores)
    w      = softmax(scores)                         # [L]
    out[b] = w @ att_feats[b]                        # [R]

Key observation: masked positions get weight exactly 0 in the reference
(exp(-1e8 - max) == 0 in fp32), so their p/f rows never matter.  The
host gathers only the kept rows per batch (l-compaction), pads to a
common LP = ceil(max_b n_keep / 128) * 128, and a keep mask kills the
padded slots.  ~Half of L is masked -> ~same fraction of DMA + compute
saved.

Sharding: data-parallel over batch B=32 across 8 cores (4 rows/core).

Layouts (host-prepared; compact index j in [0, LP), chunk n = j % nch,
partition q = j // nch, so every DMA line is one long contiguous run
per partition -> few descriptors, SDMA line rate):
  p    [bs, 128, nch, A]   bf16
  f    [bs, 128, nch, RNN] bf16
  keep [128, bs, nch]      f32    1.0 where q*nch+n < n_keep[b]
  hT   [128, kch, bs]      bf16   h.T with rnn index q*kch+c
  whT  [128, kch, A]       bf16   W_h.T with the same rnn chunking
  bh/wa [1, A]             bf16

Device program per core (software-pipelined, stage-interleaved so no
engine's FIFO queue head-of-line-blocks the next batch row):
  - all input DMAs issued up front (sync/HWDGE ring drains in order:
    consts, p0, p1, f0, p2, f1, p3, f2, f3)
  - phase 0 on PE: att_h for all 4 rows in ONE accumulation chain
    (lhsT=[128,4]); +b_h on DVE; partition-broadcast via K=1
    ones-matmuls; W_a broadcast replicated nch x (wa_bc9) so the
    phase-A multiply is a whole-group DVE op.
  - per b: adds (GPSIMD, per chunk, in place) -> tanh (ACT, 3-chunk
    groups, in place) -> *wa_bc9 (DVE, groups) -> free-dim reduce to
    scores (split DVE reduce_sum / ACT Copy+accum_out) -> exp (ACT)
    -> *keep (DVE, bf16 w) -> Z (GPSIMD XYZWC reduce) -> 1/Z (DVE).
  - phase B on PE: out[b] = w @ f[b] as lhsT=w column [128,1] bf16,
    rhs = f chunk [128,512] bf16, accumulated over nch chunks into
    per-b PSUM [1,512] x 2 (8 banks total, all live).
  - tail: res = rps * (1/Z) on DVE, DMA out.  Scales/out-DMAs live at
    the very end of each queue so nothing upstream stalls behind them.
"""

import os
import sys

import ml_dtypes
import numpy as np

sys.path.insert(0, "/opt/trn_rl_repo")

import concourse.bass as bass  # noqa: E402
import concourse.tile as tile  # noqa: E402
from concourse import bacc, bass_isa, mybir  # noqa: E402
from concourse.bass_utils import run_bass_kernel_spmd  # noqa: E402

N_CORES = 8
B, L, RNN, A = 32, 2048, 1024, 512
BS = B // N_CORES

F32 = mybir.dt.float32
BF16 = mybir.dt.bfloat16
TANH = mybir.ActivationFunctionType.Tanh
EXP = mybir.ActivationFunctionType.Exp
COPY = mybir.ActivationFunctionType.Copy

KERNEL_VERSION = 22

_RED_PAT = os.environ.get("K_RED", "vavav")  # v=DVE reduce, a=ACT accum
_Z_ON_GP = os.environ.get("K_ZGP", "1") == "1"
_PGRP = int(os.environ.get("K_PGRP", "3"))


def build_program(bs=BS, nch=9, rnn=RNN, a=A, pgrp=_PGRP,
                  red_pat=_RED_PAT, z_on_gp=_Z_ON_GP):
    kch = rnn // 128         # contraction chunks for att_h
    nh = 2                   # 512-wide output halves of phase B
    rh = rnn // nh
    ng = (nch + pgrp - 1) // pgrp
    grps = [slice(g * pgrp, min((g + 1) * pgrp, nch)) for g in range(ng)]

    nc = bacc.Bacc(None, target_bir_lowering=False)
    p = nc.dram_tensor("p", [bs, 128, nch, a], BF16, kind="ExternalInput")
    f = nc.dram_tensor("f", [bs, 128, nch, rnn], BF16, kind="ExternalInput")
    hT = nc.dram_tensor("hT", [128, kch, bs], BF16, kind="ExternalInput")
    whT = nc.dram_tensor("whT", [128, kch, a], BF16, kind="ExternalInput")
    bh = nc.dram_tensor("bh", [1, a], BF16, kind="ExternalInput")
    wa = nc.dram_tensor("wa", [1, a], BF16, kind="ExternalInput")
    keep = nc.dram_tensor("keep", [128, bs, nch], F32, kind="ExternalInput")
    # unused input whose SHAPE encodes the kernel version: the compile
    # cache keys on the HLO signature (names/shapes), NOT the embedded
    # BIR — without this, a rebuilt kernel with unchanged I/O silently
    # re-runs the previously cached NEFF.
    ver = nc.dram_tensor("ver", [KERNEL_VERSION, 1], F32, kind="ExternalInput")
    out = nc.dram_tensor("out", [bs, rnn], F32, kind="ExternalOutput")

    with tile.TileContext(nc) as tc:
        with (
            tc.tile_pool(name="singles", bufs=1) as singles,
            tc.tile_pool(name="ppool", bufs=bs) as ppool,
            tc.tile_pool(name="fpool", bufs=bs) as fpool,
            tc.tile_pool(name="sm", bufs=bs) as smpool,
            tc.tile_pool(name="respool", bufs=bs) as respool,
        ):
            # ---- constants (small, first on the DMA ring) ----
            hT_sb = singles.tile([128, kch, bs], BF16)
            nc.sync.dma_start(out=hT_sb, in_=hT[:, :, :])
            whT_sb = singles.tile([128, kch, a], BF16)
            nc.sync.dma_start(out=whT_sb, in_=whT[:, :, :])
            bh_row = singles.tile([1, a], BF16)
            nc.sync.dma_start(out=bh_row, in_=bh[:, :])
            wa_row = singles.tile([1, a], BF16)
            nc.sync.dma_start(out=wa_row, in_=wa[:, :])
            keep_sb = singles.tile([128, bs, nch], F32)
            nc.sync.dma_start(out=keep_sb, in_=keep[:, :, :])
            ver_sb = singles.tile([KERNEL_VERSION, 1], F32)
            nc.sync.dma_start(out=ver_sb, in_=ver[:, :])
            ones_col = singles.tile([128, 1], F32)
            nc.vector.memset(ones_col, 1.0)
            ones_row = singles.tile([1, 128], BF16)
            nc.vector.memset(ones_row, 1.0)
            wa_bc9 = singles.tile([128, nch, a], BF16)
            attb_bc = singles.tile([128, bs, a], BF16)

            # ---- input DMAs, issued up front in wave order ----
            ptiles = [ppool.tile([128, nch, a], BF16, tag="p", name=f"p{b}")
                      for b in range(bs)]
            ftiles = [fpool.tile([128, nch, rnn], BF16, tag="f", name=f"f{b}")
                      for b in range(bs)]

            def dma_p(b):
                for sl in grps:
                    nc.sync.dma_start(out=ptiles[b][:, sl, :],
                                      in_=p[b, :, sl, :])

            def dma_f(b):
                nc.sync.dma_start(out=ftiles[b], in_=f[b, :, :, :])

            dma_p(0)
            dma_p(1)
            dma_f(0)
            dma_p(2)
            dma_f(1)
            dma_p(3)
            dma_f(2)
            dma_f(3)

            # ---- phase 0 ----
            with tc.tile_pool(name="ps0", bufs=1, space="PSUM") as ps0:
                wa_ps = ps0.tile([128, a], F32, tag="bc")
                nc.tensor.matmul(wa_ps, lhsT=ones_row, rhs=wa_row,
                                 start=True, stop=True)
                nc.scalar.copy(out=wa_bc9[:, 0, :], in_=wa_ps)
                for j in range(1, nch):
                    # spread the replication over three queues
                    e = (nc.scalar, nc.vector, nc.gpsimd)[j % 3]
                    if e is nc.scalar:
                        e.copy(out=wa_bc9[:, j, :], in_=wa_bc9[:, 0, :])
                    else:
                        e.tensor_copy(wa_bc9[:, j, :], wa_bc9[:, 0, :])

                # per-b chains: PSUM reads / matmul operands must start
                # at partition 0, so batching b onto partitions is out
                for b in range(bs):
                    ah_ps = ps0.tile([1, a], F32, tag="ah", name=f"ah{b}")
                    for c in range(kch):
                        nc.tensor.matmul(ah_ps, lhsT=hT_sb[:, c, b:b + 1],
                                         rhs=whT_sb[:, c, :],
                                         start=(c == 0), stop=(c == kch - 1))
                    attb_row = smpool.tile([1, a], BF16, tag="attbrow",
                                           name=f"ar{b}")
                    nc.vector.tensor_add(attb_row, ah_ps, bh_row)
                    bc_ps = ps0.tile([128, a], F32, tag=f"bc{b % 2}",
                                     name=f"bc{b}")
                    nc.tensor.matmul(bc_ps, lhsT=ones_row, rhs=attb_row,
                                     start=True, stop=True)
                    nc.scalar.copy(out=attb_bc[:, b, :], in_=bc_ps)

            # ---- main pipeline (PSUM: 2 banks x 4 rows, all live) ----
            with tc.tile_pool(name="psacc", bufs=1, space="PSUM") as psacc:
                rps = [[psacc.tile([1, rh], F32, tag=f"r{b}_{hh}",
                                   name=f"rps{b}_{hh}")
                        for hh in range(nh)] for b in range(bs)]
                zps = [] if z_on_gp else [
                    psacc.tile([1, 1], F32, tag=f"z{b}", name=f"zps{b}")
                    for b in range(bs)]
                scores_t = [smpool.tile([128, nch], F32, tag="scores",
                                        name=f"sc{b}") for b in range(bs)]
                e_t = [smpool.tile([128, nch], F32, tag="e", name=f"e{b}")
                       for b in range(bs)]
                w_t = [smpool.tile([128, nch], BF16, tag="w", name=f"w{b}")
                       for b in range(bs)]
                z_t = [smpool.tile([128, 1], F32, tag="z", name=f"z{b}")
                       for b in range(bs)]
                zinv_t = [smpool.tile([1, 1], F32, tag="zi", name=f"zi{b}")
                          for b in range(bs)]

                def adds(b):
                    for j in range(nch):
                        nc.gpsimd.tensor_add(
                            ptiles[b][:, j, :], ptiles[b][:, j, :],
                            attb_bc[:, b, :])

                def tanh(b):
                    for sl in grps:
                        nc.scalar.activation(out=ptiles[b][:, sl, :],
                                             in_=ptiles[b][:, sl, :],
                                             func=TANH)

                def mulred(b):
                    for g, sl in enumerate(grps):
                        nc.vector.tensor_mul(ptiles[b][:, sl, :],
                                             ptiles[b][:, sl, :],
                                             wa_bc9[:, sl, :])
                        for j in range(sl.start, sl.stop):
                            if red_pat[j % len(red_pat)] == "v":
                                nc.vector.reduce_sum(
                                    scores_t[b][:, j:j + 1],
                                    ptiles[b][:, j, :],
                                    axis=mybir.AxisListType.X)
                            else:
                                nc.scalar.activation(
                                    out=ptiles[b][:, j, :],
                                    in_=ptiles[b][:, j, :], func=COPY,
                                    accum_out=scores_t[b][:, j:j + 1])

                def softmax(b):
                    nc.scalar.activation(out=e_t[b], in_=scores_t[b],
                                         func=EXP)
                    nc.vector.tensor_mul(w_t[b], e_t[b], keep_sb[:, b, :])
                    if z_on_gp:
                        zpart = smpool.tile([128, 1], F32, tag="zp",
                                            name=f"zp{b}")
                        nc.vector.reduce_sum(zpart, w_t[b],
                                             axis=mybir.AxisListType.X)
                        nc.gpsimd.partition_all_reduce(
                            z_t[b], zpart, channels=128,
                            reduce_op=bass_isa.ReduceOp.add)
                        nc.vector.reciprocal(zinv_t[b], z_t[b][0:1, :])
                    else:
                        zpart = smpool.tile([128, 1], F32, tag="zp",
                                            name=f"zp{b}")
                        nc.vector.reduce_sum(zpart, w_t[b],
                                             axis=mybir.AxisListType.X)
                        nc.tensor.matmul(zps[b], lhsT=ones_col, rhs=zpart,
                                         start=True, stop=True)
                        nc.vector.reciprocal(zinv_t[b], zps[b])

                def phase_b(b):
                    for j in range(nch):
                        lhs = w_t[b][:, j:j + 1]
                        for hh in range(nh):
                            nc.tensor.matmul(
                                rps[b][hh], lhsT=lhs,
                                rhs=ftiles[b][:, j, hh * rh:(hh + 1) * rh],
                                start=(j == 0), stop=(j == nch - 1))

                for b in range(bs):
                    adds(b)
                tanh(0)
                for b in range(bs):
                    if b + 1 < bs:
                        tanh(b + 1)
                    mulred(b)
                    softmax(b)
                    phase_b(b)

                # tail: scale by 1/Z and store
                for b in range(bs):
                    res = respool.tile([1, rnn], F32, tag="res",
                                       name=f"res{b}")
                    for hh in range(nh):
                        nc.vector.tensor_scalar_mul(
                            res[:, hh * rh:(hh + 1) * rh], rps[b][hh],
                            zinv_t[b])
                    nc.sync.dma_start(out=out[b:b + 1, :], in_=res)
    nc.finalize()
    return nc


_PROG = None
_PROG_NCH = None


def _get_program(nch):
    global _PROG, _PROG_NCH
    if _PROG is None or _PROG_NCH != nch:
        _PROG = build_program(nch=nch)
        _PROG_NCH = nch
    return _PROG


def make_in_maps(h, att_feats, p_att_feats, mask, W_h, b_h, W_a):
    h = np.asarray(h, dtype=np.float32)
    att_feats = np.asarray(att_feats, dtype=np.float32)
    p_att_feats = np.asarray(p_att_feats, dtype=np.float32)
    mask = np.asarray(mask)

    # l-compaction: keep only unmasked positions, pad to a common LP
    keep_idx = [np.nonzero(~mask[b])[0] for b in range(B)]
    nkeep = np.array([len(ix) for ix in keep_idx])
    LP = max(128, int(-(-nkeep.max() // 128)) * 128)
    nch = LP // 128

    bf16 = ml_dtypes.bfloat16
    kch = RNN // 128
    hT = np.ascontiguousarray(h.T.reshape(128, kch, B).astype(bf16))
    whT = np.ascontiguousarray(
        np.asarray(W_h, np.float32).T.reshape(128, kch, A).astype(bf16))
    bh = np.asarray(b_h, np.float32).reshape(1, A).astype(bf16)
    wav = np.asarray(W_a, np.float32).reshape(1, A).astype(bf16)
    ver = np.zeros((KERNEL_VERSION, 1), np.float32)

    pc = np.zeros((B, 128, nch, A), dtype=bf16)
    fc = np.zeros((B, 128, nch, RNN), dtype=bf16)
    keepm = np.zeros((B, 128, nch), dtype=np.float32)
    ar = np.arange(LP)
    for b in range(B):
        ix = keep_idx[b]
        nb = len(ix)
        pc[b] = np.concatenate(
            [p_att_feats[b][ix].astype(bf16),
             np.zeros((LP - nb, A), dtype=bf16)], axis=0
        ).reshape(128, nch, A)
        fc[b] = np.concatenate(
            [att_feats[b][ix].astype(bf16),
             np.zeros((LP - nb, RNN), dtype=bf16)], axis=0
        ).reshape(128, nch, RNN)
        keepm[b] = (ar < nb).astype(np.float32).reshape(128, nch)

    in_maps = []
    for c in range(N_CORES):
        s = slice(c * BS, (c + 1) * BS)
        in_maps.append({
            "p": np.ascontiguousarray(pc[s]),
            "f": np.ascontiguousarray(fc[s]),
            "hT": np.ascontiguousarray(hT[:, :, s]),
            "whT": whT,
            "bh": bh,
            "wa": wav,
            "keep": np.ascontiguousarray(keepm[s].transpose(1, 0, 2)),
            "ver": ver,
        })
    return in_maps, nch


def run_sharded(inputs, trace=False, **kwargs):
    in_maps, nch = make_in_maps(
        inputs["h"], inputs["att_feats"], inputs["p_att_feats"],
        inputs["mask"], inputs["W_h"], inputs["b_h"], inputs["W_a"])
    nc = _get_program(nch)
    return run_bass_kernel_spmd(nc, in_maps, core_ids=list(range(N_CORES)),
                                trace=trace, **kwargs)


def kernel(h, att_feats, p_att_feats, mask, W_h, b_h, W_a, b_a):
    res = run_sharded({
        "h": h, "att_feats": att_feats, "p_att_feats": p_att_feats,
        "mask": mask, "W_h": W_h, "b_h": b_h, "W_a": W_a, "b_a": b_a})
    return np.concatenate([res.results[c]["out"] for c in range(N_CORES)],
                          axis=0).astype(np.float32)


# revision 18
# speedup vs baseline: 2.5543x; 1.7031x over previous
"""Trainium2 Bass kernel for the Attention2 module (v3: a-on-partitions).

Computation (per batch row b):
    att_h  = h[b] @ W_h.T + b_h                      # [A]
    dot    = tanh(p_att_feats[b] + att_h)            # [L, A]
    scores = dot @ W_a[0]  (+ b_a, dropped: softmax shift-invariant)
    scores = where(mask, -1e8, scores)
    w      = softmax(scores)                         # [L]
    out[b] = w @ att_feats[b]                        # [R]

Masked positions get weight exactly 0 in the reference (exp(-1e8 - max)
== 0 in fp32), so their p/f rows never matter: the host gathers only
kept rows per batch (l-compaction), pads to LP = ceil(max n_keep/128)
* 128; a keep mask kills the padded slots.

v3 layout choice: p is transposed on the host to [A, LP] so the A
(hidden) dim sits on partitions.  Then
  - the "+ att_h" add fuses into the tanh ACTIVATE as its per-partition
    bias operand (was 47 us of GPSIMD adds in v2),
  - "scores = W_a . tanh" becomes PE matmuls contracting over the
    partition dim (was DVE multiply + free-dim reduce),
leaving the Vector/GpSimd engines nearly empty and ACT with just tanh.

Sharding: data-parallel over batch B=32 across 8 cores (4 rows/core).

Host layouts (a = q*4 + c, rnn = q*8 + c so every DMA line is one
contiguous run per partition; l keeps chunk-major n*128 + q to match
the PE-transpose output ordering):
  pT   [bs, 128, 4, LP]      bf16   pT[b,q,c,:] = p[b, kept_l, q*4+c].T
  f    [bs, 128, nch, RNN]   bf16   f[b,q,n]    = att[b, kept n*128+q]
  hT   [128, kch, bs]        bf16   h.T, rnn index q*8+c
  whT4 [128, kch, 4, 128]    bf16   whT4[k,kc,c,q] = W_h.T[...,q*4+c]
  bhT/waT [128, 4]           f32/bf16 (a = q*4+c)
  keep [128, bs, nch]        f32    l = n*128+q

Device program per core:
  phase 0 on PE: att_h directly in [a-partition, b-free] orientation —
    lhsT = whT4 chunk [128,128], rhs = hT chunk [128,4], accumulate
    over kch; + b_h via DVE tensor_scalar (per-partition bias).
  per b: tanh(pT*1 + attbT) on ACT (bias fused);
    scores quarters [1,<=512] on PE (lhsT = waT column, rhs = tanh
    tile, accumulated over the 4 a-chunks);
    exp PSUM->SBUF row [1,LP] bf16 on ACT;
    PE transposes [1,128] -> [128,1] to give w with l on partitions;
    * keep + Z (DVE row-sum + GPSIMD partition_all_reduce) off the
    critical path; phase B as before: lhsT = w column [128,1] bf16,
    rhs = f chunk [128,512] bf16 -> PSUM [1,512] x 2 per b;
    scale by 1/Z on DVE, DMA out.
"""

import os
import sys

import ml_dtypes
import numpy as np

sys.path.insert(0, "/opt/trn_rl_repo")

import concourse.bass as bass  # noqa: E402
import concourse.tile as tile  # noqa: E402
from concourse import bacc, bass_isa, mybir  # noqa: E402
from concourse.bass_utils import run_bass_kernel_spmd  # noqa: E402

N_CORES = 8
B, L, RNN, A = 32, 2048, 1024, 512
BS = B // N_CORES

F32 = mybir.dt.float32
BF16 = mybir.dt.bfloat16
TANH = mybir.ActivationFunctionType.Tanh
EXP = mybir.ActivationFunctionType.Exp

KERNEL_VERSION = 31

ACH = A // 128  # a-chunks (4)


def build_program(bs=BS, nch=9, rnn=RNN, a=A):
    kch = rnn // 128         # contraction chunks for att_h
    nh = 2                   # 512-wide output halves of phase B
    rh = rnn // nh
    lp = nch * 128
    # score quarters: [1, <=512] PSUM tiles covering LP
    nq = (lp + 511) // 512
    qsl = [slice(q * 512, min((q + 1) * 512, lp)) for q in range(nq)]
    # PSUM banks: nq (scores) + 1 (wT) + 2*rps_bufs (phase B) <= 8
    rps_bufs = max(1, (8 - nq - 1) // 2)

    nc = bacc.Bacc(None, target_bir_lowering=False)
    p = nc.dram_tensor("p", [bs, 128, ACH, lp], BF16, kind="ExternalInput")
    f = nc.dram_tensor("f", [bs, 128, nch, rnn], BF16, kind="ExternalInput")
    hT = nc.dram_tensor("hT", [128, kch, bs], BF16, kind="ExternalInput")
    whT4 = nc.dram_tensor("whT4", [128, kch, ACH, 128], BF16,
                          kind="ExternalInput")
    bhT = nc.dram_tensor("bhT", [128, ACH], F32, kind="ExternalInput")
    waT = nc.dram_tensor("waT", [128, ACH], BF16, kind="ExternalInput")
    keep = nc.dram_tensor("keep", [128, bs, nch], F32, kind="ExternalInput")
    # unused input whose SHAPE encodes the kernel version: the compile
    # cache keys on the HLO signature (names/shapes), NOT the embedded
    # BIR — without this, a rebuilt kernel with unchanged I/O silently
    # re-runs the previously cached NEFF.
    ver = nc.dram_tensor("ver", [KERNEL_VERSION, 1], F32, kind="ExternalInput")
    out = nc.dram_tensor("out", [bs, rnn], F32, kind="ExternalOutput")

    with tile.TileContext(nc) as tc:
        with (
            tc.tile_pool(name="singles", bufs=1) as singles,
            tc.tile_pool(name="ppool", bufs=bs) as ppool,
            tc.tile_pool(name="fpool", bufs=bs) as fpool,
            tc.tile_pool(name="sm", bufs=bs) as smpool,
            tc.tile_pool(name="respool", bufs=bs) as respool,
        ):
            # ---- constants (small, first on the DMA ring) ----
            hT_sb = singles.tile([128, kch, bs], BF16)
            nc.sync.dma_start(out=hT_sb, in_=hT[:, :, :])
            whT4_sb = singles.tile([128, kch, ACH, 128], BF16)
            nc.sync.dma_start(out=whT4_sb, in_=whT4[:, :, :, :])
            bhT_sb = singles.tile([128, ACH], F32)
            nc.sync.dma_start(out=bhT_sb, in_=bhT[:, :])
            waT_sb = singles.tile([128, ACH], BF16)
            nc.sync.dma_start(out=waT_sb, in_=waT[:, :])
            keep_sb = singles.tile([128, bs, nch], F32)
            nc.sync.dma_start(out=keep_sb, in_=keep[:, :, :])
            ver_sb = singles.tile([KERNEL_VERSION, 1], F32)
            nc.sync.dma_start(out=ver_sb, in_=ver[:, :])
            ident = singles.tile([1, 1], F32)
            nc.vector.memset(ident, 1.0)
            attbT = singles.tile([128, ACH, bs], F32)

            # ---- input DMAs, issued up front in wave order ----
            ptiles = [ppool.tile([128, ACH, lp], BF16, tag="p", name=f"p{b}")
                      for b in range(bs)]
            ftiles = [fpool.tile([128, nch, rnn], BF16, tag="f", name=f"f{b}")
                      for b in range(bs)]

            def dma_p(b):
                for c in range(0, ACH, 2):
                    nc.sync.dma_start(out=ptiles[b][:, c:c + 2, :],
                                      in_=p[b, :, c:c + 2, :])

            def dma_f(b):
                nc.sync.dma_start(out=ftiles[b], in_=f[b, :, :, :])

            dma_p(0)
            dma_p(1)
            dma_f(0)
            dma_p(2)
            dma_f(1)
            dma_p(3)
            dma_f(2)
            dma_f(3)

            # ---- phase 0: attbT[aq, c, b] = (h @ W_h.T + b_h).T ----
            with tc.tile_pool(name="ps0", bufs=1, space="PSUM") as ps0:
                for c in range(ACH):
                    at_ps = ps0.tile([128, bs], F32, tag=f"at{c % 2}",
                                     name=f"at{c}")
                    for k in range(kch):
                        nc.tensor.matmul(at_ps, lhsT=whT4_sb[:, k, c, :],
                                         rhs=hT_sb[:, k, :],
                                         start=(k == 0), stop=(k == kch - 1))
                    nc.vector.tensor_scalar_add(attbT[:, c, :], at_ps,
                                                bhT_sb[:, c:c + 1])

            # ---- main pipeline ----
            with tc.tile_pool(name="pssc", bufs=1, space="PSUM") as pssc, \
                 tc.tile_pool(name="pswt", bufs=1, space="PSUM") as pswt, \
                 tc.tile_pool(name="psacc", bufs=rps_bufs,
                              space="PSUM") as psacc:
                e_t = [smpool.tile([1, lp], F32, tag="e", name=f"e{b}")
                       for b in range(bs)]
                w_t = [smpool.tile([128, nch], BF16, tag="w", name=f"w{b}")
                       for b in range(bs)]
                z_t = [smpool.tile([128, 1], F32, tag="z", name=f"z{b}")
                       for b in range(bs)]
                zinv_t = [smpool.tile([1, 1], F32, tag="zi", name=f"zi{b}")
                          for b in range(bs)]

                def tanh(b):
                    for c in range(ACH):
                        nc.scalar.activation(out=ptiles[b][:, c, :],
                                             in_=ptiles[b][:, c, :],
                                             func=TANH,
                                             bias=attbT[:, c, b:b + 1])

                def scores(b):
                    sc = [pssc.tile([1, s.stop - s.start], F32, tag=f"sq{q}",
                                    name=f"sc{b}_{q}")
                          for q, s in enumerate(qsl)]
                    for q, s in enumerate(qsl):
                        for c in range(ACH):
                            nc.tensor.matmul(sc[q], lhsT=waT_sb[:, c:c + 1],
                                             rhs=ptiles[b][:, c, s],
                                             start=(c == 0),
                                             stop=(c == ACH - 1))
                    for q, s in enumerate(qsl):
                        nc.scalar.activation(out=e_t[b][:, s], in_=sc[q],
                                             func=EXP)

                def make_w(b):
                    wt_ps = pswt.tile([128, nch], F32, tag="wt",
                                      name=f"wt{b}")
                    for s in range(nch):
                        nc.tensor.transpose(
                            wt_ps[:, s:s + 1],
                            e_t[b][:, s * 128:(s + 1) * 128], ident)
                    nc.vector.tensor_mul(w_t[b], wt_ps, keep_sb[:, b, :])
                    zpart = smpool.tile([128, 1], F32, tag="zp",
                                        name=f"zp{b}")
                    nc.vector.reduce_sum(zpart, w_t[b],
                                         axis=mybir.AxisListType.X)
                    nc.gpsimd.partition_all_reduce(
                        z_t[b], zpart, channels=128,
                        reduce_op=bass_isa.ReduceOp.add)
                    nc.vector.reciprocal(zinv_t[b], z_t[b][0:1, :])

                def phase_b(b, rps):
                    for j in range(nch):
                        lhs = w_t[b][:, j:j + 1]
                        for hh in range(nh):
                            nc.tensor.matmul(
                                rps[hh], lhsT=lhs,
                                rhs=ftiles[b][:, j, hh * rh:(hh + 1) * rh],
                                start=(j == 0), stop=(j == nch - 1))

                def scale_out(b, rps):
                    res = respool.tile([1, rnn], F32, tag="res",
                                       name=f"res{b}")
                    for hh in range(nh):
                        nc.vector.tensor_scalar_mul(
                            res[:, hh * rh:(hh + 1) * rh], rps[hh],
                            zinv_t[b])
                    nc.sync.dma_start(out=out[b:b + 1, :], in_=res)

                pending = []
                for b in range(bs):
                    tanh(b)
                    scores(b)
                    make_w(b)
                    # free the PSUM slot this b will reuse first
                    while len(pending) >= rps_bufs:
                        bb, rr = pending.pop(0)
                        scale_out(bb, rr)
                    rps = [psacc.tile([1, rh], F32, tag=f"r{hh}",
                                      name=f"rps{b}_{hh}")
                           for hh in range(nh)]
                    phase_b(b, rps)
                    pending.append((b, rps))
                for bb, rr in pending:
                    scale_out(bb, rr)
    nc.finalize()
    return nc


_PROG = None
_PROG_NCH = None


def _get_program(nch):
    global _PROG, _PROG_NCH
    if _PROG is None or _PROG_NCH != nch:
        _PROG = build_program(nch=nch)
        _PROG_NCH = nch
    return _PROG


def make_in_maps(h, att_feats, p_att_feats, mask, W_h, b_h, W_a):
    h = np.asarray(h, dtype=np.float32)
    att_feats = np.asarray(att_feats, dtype=np.float32)
    p_att_feats = np.asarray(p_att_feats, dtype=np.float32)
    mask = np.asarray(mask)

    # l-compaction: keep only unmasked positions, pad to a common LP
    keep_idx = [np.nonzero(~mask[b])[0] for b in range(B)]
    nkeep = np.array([len(ix) for ix in keep_idx])
    LP = max(128, int(-(-nkeep.max() // 128)) * 128)
    nch = LP // 128

    bf16 = ml_dtypes.bfloat16
    kch = RNN // 128
    # rnn index = q*kch + c  <=>  plain reshape(128, kch)
    hT = np.ascontiguousarray(h.T.reshape(128, kch, B).astype(bf16))
    # whT4[kq, kc, c, aq] = W_h.T[kq*kch+kc, aq*ACH+c]
    whT4 = np.ascontiguousarray(
        np.asarray(W_h, np.float32).T.reshape(128, kch, 128, ACH)
        .transpose(0, 1, 3, 2).astype(bf16))
    bhT = np.ascontiguousarray(
        np.asarray(b_h, np.float32).reshape(128, ACH))
    waT = np.ascontiguousarray(
        np.asarray(W_a, np.float32).reshape(128, ACH).astype(bf16))
    ver = np.zeros((KERNEL_VERSION, 1), np.float32)

    pc = np.zeros((B, 128, ACH, LP), dtype=bf16)
    fc = np.zeros((B, 128, nch, RNN), dtype=bf16)
    keepm = np.zeros((B, 128, nch), dtype=np.float32)
    ar = np.arange(LP)
    for b in range(B):
        ix = keep_idx[b]
        nb = len(ix)
        # pT: [A, LP] with a = q*ACH + c
        pt = np.zeros((A, LP), dtype=bf16)
        pt[:, :nb] = p_att_feats[b][ix].T.astype(bf16)
        pc[b] = pt.reshape(128, ACH, LP)
        # f: l = n*128 + q (chunk-major, matches transpose output order)
        fr = np.zeros((LP, RNN), dtype=bf16)
        fr[:nb] = att_feats[b][ix].astype(bf16)
        fc[b] = fr.reshape(nch, 128, RNN).transpose(1, 0, 2)
        keepm[b] = (ar < nb).astype(np.float32).reshape(nch, 128).T

    in_maps = []
    for c in range(N_CORES):
        s = slice(c * BS, (c + 1) * BS)
        in_maps.append({
            "p": np.ascontiguousarray(pc[s]),
            "f": np.ascontiguousarray(fc[s]),
            "hT": np.ascontiguousarray(hT[:, :, s]),
            "whT4": whT4,
            "bhT": bhT,
            "waT": waT,
            "keep": np.ascontiguousarray(keepm[s].transpose(1, 0, 2)),
            "ver": ver,
        })
    return in_maps, nch


def run_sharded(inputs, trace=False, **kwargs):
    in_maps, nch = make_in_maps(
        inputs["h"], inputs["att_feats"], inputs["p_att_feats"],
        inputs["mask"], inputs["W_h"], inputs["b_h"], inputs["W_a"])
    nc = _get_program(nch)
    return run_bass_kernel_spmd(nc, in_maps, core_ids=list(range(N_CORES)),
                                trace=trace, **kwargs)


def kernel(h, att_feats, p_att_feats, mask, W_h, b_h, W_a, b_a):
    res = run_sharded({
        "h": h, "att_feats": att_feats, "p_att_feats": p_att_feats,
        "mask": mask, "W_h": W_h, "b_h": b_h, "W_a": W_a, "b_a": b_a})
    return np.concatenate([res.results[c]["out"] for c in range(N_CORES)],
                          axis=0).astype(np.float32)


# revision 20
# speedup vs baseline: 2.7200x; 1.0649x over previous
"""Trainium2 Bass kernel for the Attention2 module (v3: a-on-partitions).

Computation (per batch row b):
    att_h  = h[b] @ W_h.T + b_h                      # [A]
    dot    = tanh(p_att_feats[b] + att_h)            # [L, A]
    scores = dot @ W_a[0]  (+ b_a, dropped: softmax shift-invariant)
    scores = where(mask, -1e8, scores)
    w      = softmax(scores)                         # [L]
    out[b] = w @ att_feats[b]                        # [R]

Masked positions get weight exactly 0 in the reference (exp(-1e8 - max)
== 0 in fp32), so their p/f rows never matter: the host gathers only
kept rows per batch (l-compaction), pads to LP = ceil(max n_keep/128)
* 128; a keep mask kills the padded slots.

v3 layout choice: p is transposed on the host to [A, LP] so the A
(hidden) dim sits on partitions.  Then
  - the "+ att_h" add fuses into the tanh ACTIVATE as its per-partition
    bias operand (was 47 us of GPSIMD adds in v2),
  - "scores = W_a . tanh" becomes PE matmuls contracting over the
    partition dim (was DVE multiply + free-dim reduce),
leaving the Vector/GpSimd engines nearly empty and ACT with just tanh.

Sharding: data-parallel over batch B=32 across 8 cores (4 rows/core).

Host layouts (a = q*4 + c, rnn = q*8 + c so every DMA line is one
contiguous run per partition; l keeps chunk-major n*128 + q to match
the PE-transpose output ordering):
  pT   [bs, 128, 4, LP]      bf16   pT[b,q,c,:] = p[b, kept_l, q*4+c].T
  f    [bs, 128, nch, RNN]   bf16   f[b,q,n]    = att[b, kept n*128+q]
  hT   [128, kch, bs]        bf16   h.T, rnn index q*8+c
  whT4 [128, kch, 4, 128]    bf16   whT4[k,kc,c,q] = W_h.T[...,q*4+c]
  bhT/waT [128, 4]           f32/bf16 (a = q*4+c)
  keep [128, bs, nch]        f32    l = n*128+q

Device program per core:
  phase 0 on PE: att_h directly in [a-partition, b-free] orientation —
    lhsT = whT4 chunk [128,128], rhs = hT chunk [128,4], accumulate
    over kch; + b_h via DVE tensor_scalar (per-partition bias).
  per b: tanh(pT*1 + attbT) on ACT (bias fused);
    scores quarters [1,<=512] on PE (lhsT = waT column, rhs = tanh
    tile, accumulated over the 4 a-chunks);
    exp PSUM->SBUF row [1,LP] bf16 on ACT;
    PE transposes [1,128] -> [128,1] to give w with l on partitions;
    * keep + Z (DVE row-sum + GPSIMD partition_all_reduce) off the
    critical path; phase B as before: lhsT = w column [128,1] bf16,
    rhs = f chunk [128,512] bf16 -> PSUM [1,512] x 2 per b;
    scale by 1/Z on DVE, DMA out.
"""

import os
import sys

import ml_dtypes
import numpy as np

sys.path.insert(0, "/opt/trn_rl_repo")

import concourse.bass as bass  # noqa: E402
import concourse.tile as tile  # noqa: E402
from concourse import bacc, bass_isa, mybir  # noqa: E402
from concourse.bass_utils import run_bass_kernel_spmd  # noqa: E402

N_CORES = 8
B, L, RNN, A = 32, 2048, 1024, 512
BS = B // N_CORES

F32 = mybir.dt.float32
BF16 = mybir.dt.bfloat16
TANH = mybir.ActivationFunctionType.Tanh
EXP = mybir.ActivationFunctionType.Exp

KERNEL_VERSION = 32

ACH = A // 128  # a-chunks (4)


def build_program(bs=BS, nch=9, rnn=RNN, a=A):
    kch = rnn // 128         # contraction chunks for att_h
    nh = 2                   # 512-wide output halves of phase B
    rh = rnn // nh
    lp = nch * 128
    # score quarters: [1, <=512] PSUM tiles covering LP
    nq = (lp + 511) // 512
    qsl = [slice(q * 512, min((q + 1) * 512, lp)) for q in range(nq)]
    # PSUM banks: nq (scores) + 1 (wT) + 2*rps_bufs (phase B) <= 8
    rps_bufs = max(1, (8 - nq - 1) // 2)

    nc = bacc.Bacc(None, target_bir_lowering=False)
    p = nc.dram_tensor("p", [bs, 128, ACH, lp], BF16, kind="ExternalInput")
    f = nc.dram_tensor("f", [bs, 128, nch, rnn], BF16, kind="ExternalInput")
    hT = nc.dram_tensor("hT", [128, kch, bs], BF16, kind="ExternalInput")
    whT4 = nc.dram_tensor("whT4", [128, kch, ACH, 128], BF16,
                          kind="ExternalInput")
    bhT = nc.dram_tensor("bhT", [128, ACH], F32, kind="ExternalInput")
    waT = nc.dram_tensor("waT", [128, ACH], BF16, kind="ExternalInput")
    keep = nc.dram_tensor("keep", [128, bs, nch], F32, kind="ExternalInput")
    # unused input whose SHAPE encodes the kernel version: the compile
    # cache keys on the HLO signature (names/shapes), NOT the embedded
    # BIR — without this, a rebuilt kernel with unchanged I/O silently
    # re-runs the previously cached NEFF.
    ver = nc.dram_tensor("ver", [KERNEL_VERSION, 1], F32, kind="ExternalInput")
    out = nc.dram_tensor("out", [bs, rnn], F32, kind="ExternalOutput")

    with tile.TileContext(nc) as tc:
        with (
            tc.tile_pool(name="singles", bufs=1) as singles,
            tc.tile_pool(name="ppool", bufs=bs) as ppool,
            tc.tile_pool(name="fpool", bufs=bs) as fpool,
            tc.tile_pool(name="sm", bufs=bs) as smpool,
            tc.tile_pool(name="respool", bufs=bs) as respool,
        ):
            # ---- constants (small, first on the DMA ring) ----
            hT_sb = singles.tile([128, kch, bs], BF16)
            nc.sync.dma_start(out=hT_sb, in_=hT[:, :, :])
            whT4_sb = singles.tile([128, kch, ACH, 128], BF16)
            nc.sync.dma_start(out=whT4_sb, in_=whT4[:, :, :, :])
            bhT_sb = singles.tile([128, ACH], F32)
            nc.sync.dma_start(out=bhT_sb, in_=bhT[:, :])
            waT_sb = singles.tile([128, ACH], BF16)
            nc.sync.dma_start(out=waT_sb, in_=waT[:, :])
            keep_sb = singles.tile([128, bs, nch], F32)
            nc.sync.dma_start(out=keep_sb, in_=keep[:, :, :])
            ver_sb = singles.tile([KERNEL_VERSION, 1], F32)
            nc.sync.dma_start(out=ver_sb, in_=ver[:, :])
            ident = singles.tile([1, 1], F32)
            nc.vector.memset(ident, 1.0)
            attbT = singles.tile([128, ACH, bs], F32)

            # ---- input DMAs, issued up front in wave order ----
            ptiles = [ppool.tile([128, ACH, lp], BF16, tag="p", name=f"p{b}")
                      for b in range(bs)]
            ftiles = [fpool.tile([128, nch, rnn], BF16, tag="f", name=f"f{b}")
                      for b in range(bs)]

            def dma_p(b):
                for c in range(0, ACH, 2):
                    nc.sync.dma_start(out=ptiles[b][:, c:c + 2, :],
                                      in_=p[b, :, c:c + 2, :])

            def dma_f(b):
                # 3-chunk groups so phase B can start on the early
                # chunks while the rest of f[b] is still on the wire
                for g in range(0, nch, 3):
                    sl = slice(g, min(g + 3, nch))
                    nc.sync.dma_start(out=ftiles[b][:, sl, :],
                                      in_=f[b, :, sl, :])

            dma_p(0)
            dma_p(1)
            dma_f(0)
            dma_p(2)
            dma_f(1)
            dma_p(3)
            dma_f(2)
            dma_f(3)

            # ---- phase 0: attbT[aq, c, b] = (h @ W_h.T + b_h).T ----
            with tc.tile_pool(name="ps0", bufs=1, space="PSUM") as ps0:
                for c in range(ACH):
                    at_ps = ps0.tile([128, bs], F32, tag=f"at{c % 2}",
                                     name=f"at{c}")
                    for k in range(kch):
                        nc.tensor.matmul(at_ps, lhsT=whT4_sb[:, k, c, :],
                                         rhs=hT_sb[:, k, :],
                                         start=(k == 0), stop=(k == kch - 1))
                    nc.vector.tensor_scalar_add(attbT[:, c, :], at_ps,
                                                bhT_sb[:, c:c + 1])

            # ---- main pipeline ----
            with tc.tile_pool(name="pssc", bufs=1, space="PSUM") as pssc, \
                 tc.tile_pool(name="pswt", bufs=1, space="PSUM") as pswt, \
                 tc.tile_pool(name="psacc", bufs=rps_bufs,
                              space="PSUM") as psacc:
                e_t = [smpool.tile([1, lp], F32, tag="e", name=f"e{b}")
                       for b in range(bs)]
                w_t = [smpool.tile([128, nch], BF16, tag="w", name=f"w{b}")
                       for b in range(bs)]
                z_t = [smpool.tile([128, 1], F32, tag="z", name=f"z{b}")
                       for b in range(bs)]
                zinv_t = [smpool.tile([1, 1], F32, tag="zi", name=f"zi{b}")
                          for b in range(bs)]

                def tanh(b):
                    for c in range(ACH):
                        nc.scalar.activation(out=ptiles[b][:, c, :],
                                             in_=ptiles[b][:, c, :],
                                             func=TANH,
                                             bias=attbT[:, c, b:b + 1])

                def scores(b):
                    sc = [pssc.tile([1, s.stop - s.start], F32, tag=f"sq{q}",
                                    name=f"sc{b}_{q}")
                          for q, s in enumerate(qsl)]
                    for q, s in enumerate(qsl):
                        for c in range(ACH):
                            nc.tensor.matmul(sc[q], lhsT=waT_sb[:, c:c + 1],
                                             rhs=ptiles[b][:, c, s],
                                             start=(c == 0),
                                             stop=(c == ACH - 1))
                    for q, s in enumerate(qsl):
                        nc.scalar.activation(out=e_t[b][:, s], in_=sc[q],
                                             func=EXP)

                def make_w(b):
                    wt_ps = pswt.tile([128, nch], F32, tag="wt",
                                      name=f"wt{b}")
                    for s in range(nch):
                        nc.tensor.transpose(
                            wt_ps[:, s:s + 1],
                            e_t[b][:, s * 128:(s + 1) * 128], ident)
                    nc.vector.tensor_mul(w_t[b], wt_ps, keep_sb[:, b, :])
                    zpart = smpool.tile([128, 1], F32, tag="zp",
                                        name=f"zp{b}")
                    nc.vector.reduce_sum(zpart, w_t[b],
                                         axis=mybir.AxisListType.X)
                    nc.gpsimd.partition_all_reduce(
                        z_t[b], zpart, channels=128,
                        reduce_op=bass_isa.ReduceOp.add)
                    nc.vector.reciprocal(zinv_t[b], z_t[b][0:1, :])

                def phase_b(b, rps):
                    for j in range(nch):
                        lhs = w_t[b][:, j:j + 1]
                        for hh in range(nh):
                            nc.tensor.matmul(
                                rps[hh], lhsT=lhs,
                                rhs=ftiles[b][:, j, hh * rh:(hh + 1) * rh],
                                start=(j == 0), stop=(j == nch - 1))

                def scale_out(b, rps):
                    res = respool.tile([1, rnn], F32, tag="res",
                                       name=f"res{b}")
                    for hh in range(nh):
                        nc.vector.tensor_scalar_mul(
                            res[:, hh * rh:(hh + 1) * rh], rps[hh],
                            zinv_t[b])
                    nc.sync.dma_start(out=out[b:b + 1, :], in_=res)

                pending = []
                for b in range(bs):
                    tanh(b)
                    scores(b)
                    make_w(b)
                    # free the PSUM slot this b will reuse first
                    while len(pending) >= rps_bufs:
                        bb, rr = pending.pop(0)
                        scale_out(bb, rr)
                    rps = [psacc.tile([1, rh], F32, tag=f"r{hh}",
                                      name=f"rps{b}_{hh}")
                           for hh in range(nh)]
                    phase_b(b, rps)
                    pending.append((b, rps))
                for bb, rr in pending:
                    scale_out(bb, rr)
    nc.finalize()
    return nc


_PROG = None
_PROG_NCH = None


def _get_program(nch):
    global _PROG, _PROG_NCH
    if _PROG is None or _PROG_NCH != nch:
        _PROG = build_program(nch=nch)
        _PROG_NCH = nch
    return _PROG


def make_in_maps(h, att_feats, p_att_feats, mask, W_h, b_h, W_a):
    h = np.asarray(h, dtype=np.float32)
    att_feats = np.asarray(att_feats, dtype=np.float32)
    p_att_feats = np.asarray(p_att_feats, dtype=np.float32)
    mask = np.asarray(mask)

    # l-compaction: keep only unmasked positions, pad to a common LP
    keep_idx = [np.nonzero(~mask[b])[0] for b in range(B)]
    nkeep = np.array([len(ix) for ix in keep_idx])
    LP = max(128, int(-(-nkeep.max() // 128)) * 128)
    nch = LP // 128

    bf16 = ml_dtypes.bfloat16
    kch = RNN // 128
    # rnn index = q*kch + c  <=>  plain reshape(128, kch)
    hT = np.ascontiguousarray(h.T.reshape(128, kch, B).astype(bf16))
    # whT4[kq, kc, c, aq] = W_h.T[kq*kch+kc, aq*ACH+c]
    whT4 = np.ascontiguousarray(
        np.asarray(W_h, np.float32).T.reshape(128, kch, 128, ACH)
        .transpose(0, 1, 3, 2).astype(bf16))
    bhT = np.ascontiguousarray(
        np.asarray(b_h, np.float32).reshape(128, ACH))
    waT = np.ascontiguousarray(
        np.asarray(W_a, np.float32).reshape(128, ACH).astype(bf16))
    ver = np.zeros((KERNEL_VERSION, 1), np.float32)

    pc = np.zeros((B, 128, ACH, LP), dtype=bf16)
    fc = np.zeros((B, 128, nch, RNN), dtype=bf16)
    keepm = np.zeros((B, 128, nch), dtype=np.float32)
    ar = np.arange(LP)
    for b in range(B):
        ix = keep_idx[b]
        nb = len(ix)
        # pT: [A, LP] with a = q*ACH + c
        pt = np.zeros((A, LP), dtype=bf16)
        pt[:, :nb] = p_att_feats[b][ix].T.astype(bf16)
        pc[b] = pt.reshape(128, ACH, LP)
        # f: l = n*128 + q (chunk-major, matches transpose output order)
        fr = np.zeros((LP, RNN), dtype=bf16)
        fr[:nb] = att_feats[b][ix].astype(bf16)
        fc[b] = fr.reshape(nch, 128, RNN).transpose(1, 0, 2)
        keepm[b] = (ar < nb).astype(np.float32).reshape(nch, 128).T

    in_maps = []
    for c in range(N_CORES):
        s = slice(c * BS, (c + 1) * BS)
        in_maps.append({
            "p": np.ascontiguousarray(pc[s]),
            "f": np.ascontiguousarray(fc[s]),
            "hT": np.ascontiguousarray(hT[:, :, s]),
            "whT4": whT4,
            "bhT": bhT,
            "waT": waT,
            "keep": np.ascontiguousarray(keepm[s].transpose(1, 0, 2)),
            "ver": ver,
        })
    return in_maps, nch


def run_sharded(inputs, trace=False, **kwargs):
    in_maps, nch = make_in_maps(
        inputs["h"], inputs["att_feats"], inputs["p_att_feats"],
        inputs["mask"], inputs["W_h"], inputs["b_h"], inputs["W_a"])
    nc = _get_program(nch)
    return run_bass_kernel_spmd(nc, in_maps, core_ids=list(range(N_CORES)),
                                trace=trace, **kwargs)


def kernel(h, att_feats, p_att_feats, mask, W_h, b_h, W_a, b_a):
    res = run_sharded({
        "h": h, "att_feats": att_feats, "p_att_feats": p_att_feats,
        "mask": mask, "W_h": W_h, "b_h": b_h, "W_a": W_a, "b_a": b_a})
    return np.concatenate([res.results[c]["out"] for c in range(N_CORES)],
                          axis=0).astype(np.float32)


# revision 28
# speedup vs baseline: 2.8705x; 1.0553x over previous
"""Trainium2 Bass kernel for the Attention2 module (v3: a-on-partitions).

Computation (per batch row b):
    att_h  = h[b] @ W_h.T + b_h                      # [A]
    dot    = tanh(p_att_feats[b] + att_h)            # [L, A]
    scores = dot @ W_a[0]  (+ b_a, dropped: softmax shift-invariant)
    scores = where(mask, -1e8, scores)
    w      = softmax(scores)                         # [L]
    out[b] = w @ att_feats[b]                        # [R]

Masked positions get weight exactly 0 in the reference (exp(-1e8 - max)
== 0 in fp32), so their p/f rows never matter: the host gathers only
kept rows per batch (l-compaction), pads to LP = ceil(max n_keep/128)
* 128; a keep mask kills the padded slots.

v3 layout choice: p is transposed on the host to [A, LP] so the A
(hidden) dim sits on partitions.  Then
  - the "+ att_h" add fuses into the tanh ACTIVATE as its per-partition
    bias operand (was 47 us of GPSIMD adds in v2),
  - "scores = W_a . tanh" becomes PE matmuls contracting over the
    partition dim (was DVE multiply + free-dim reduce),
leaving the Vector/GpSimd engines nearly empty and ACT with just tanh.

Sharding: data-parallel over batch B=32 across 8 cores (4 rows/core).

Host layouts (a = q*4 + c, rnn = q*8 + c so every DMA line is one
contiguous run per partition; l keeps chunk-major n*128 + q to match
the PE-transpose output ordering):
  pT   [bs, 128, 4, LP]      bf16   pT[b,q,c,:] = p[b, kept_l, q*4+c].T
  f    [bs, 128, nch, RNN]   bf16   f[b,q,n]    = att[b, kept n*128+q]
  hT   [128, kch, bs]        bf16   h.T, rnn index q*8+c
  whT4 [128, kch, 4, 128]    bf16   whT4[k,kc,c,q] = W_h.T[...,q*4+c]
  bhT/waT [128, 4]           f32/bf16 (a = q*4+c)
  keep [128, bs, nch]        f32    l = n*128+q

Device program per core:
  phase 0 on PE: att_h directly in [a-partition, b-free] orientation —
    lhsT = whT4 chunk [128,128], rhs = hT chunk [128,4], accumulate
    over kch; + b_h via DVE tensor_scalar (per-partition bias).
  per b: tanh(pT*1 + attbT) on ACT (bias fused);
    scores quarters [1,<=512] on PE (lhsT = waT column, rhs = tanh
    tile, accumulated over the 4 a-chunks);
    exp PSUM->SBUF row [1,LP] bf16 on ACT;
    PE transposes [1,128] -> [128,1] to give w with l on partitions;
    * keep + Z (DVE row-sum + GPSIMD partition_all_reduce) off the
    critical path; phase B as before: lhsT = w column [128,1] bf16,
    rhs = f chunk [128,512] bf16 -> PSUM [1,512] x 2 per b;
    scale by 1/Z on DVE, DMA out.
"""

import os
import sys

import ml_dtypes
import numpy as np

sys.path.insert(0, "/opt/trn_rl_repo")

import concourse.bass as bass  # noqa: E402
import concourse.tile as tile  # noqa: E402
from concourse import bacc, bass_isa, mybir  # noqa: E402
from concourse.bass_utils import run_bass_kernel_spmd  # noqa: E402

N_CORES = 8
B, L, RNN, A = 32, 2048, 1024, 512
BS = B // N_CORES

F32 = mybir.dt.float32
BF16 = mybir.dt.bfloat16
FP8E3 = mybir.dt.float8e3   # e3m4: p (values ~N(0,1), softmax cancels)
FP8E4 = mybir.dt.float8e4   # e4m3: W_h.T (stationary fp8-weights mode)
TANH = mybir.ActivationFunctionType.Tanh
EXP = mybir.ActivationFunctionType.Exp

KERNEL_VERSION = 33

ACH = A // 128  # a-chunks (4)


def build_program(bs=BS, nch=9, rnn=RNN, a=A):
    kch = rnn // 128         # contraction chunks for att_h
    nh = 2                   # 512-wide output halves of phase B
    rh = rnn // nh
    lp = nch * 128
    # score quarters: [1, <=512] PSUM tiles covering LP
    nq = (lp + 511) // 512
    qsl = [slice(q * 512, min((q + 1) * 512, lp)) for q in range(nq)]
    # PSUM banks: nq (scores) + 1 (wT) + 2*rps_bufs (phase B) <= 8
    rps_bufs = max(1, (8 - nq - 1) // 2)

    nc = bacc.Bacc(None, target_bir_lowering=False)
    p = nc.dram_tensor("p", [bs, 128, ACH, lp], FP8E3, kind="ExternalInput")
    f = nc.dram_tensor("f", [bs, 128, nch, rnn], BF16, kind="ExternalInput")
    hT = nc.dram_tensor("hT", [128, kch, bs], BF16, kind="ExternalInput")
    whT4 = nc.dram_tensor("whT4", [128, kch, ACH, 128], FP8E4,
                          kind="ExternalInput")
    bhT = nc.dram_tensor("bhT", [128, ACH], F32, kind="ExternalInput")
    waT = nc.dram_tensor("waT", [128, ACH], BF16, kind="ExternalInput")
    keep = nc.dram_tensor("keep", [128, bs, nch], F32, kind="ExternalInput")
    # unused input whose SHAPE encodes the kernel version: the compile
    # cache keys on the HLO signature (names/shapes), NOT the embedded
    # BIR — without this, a rebuilt kernel with unchanged I/O silently
    # re-runs the previously cached NEFF.
    ver = nc.dram_tensor("ver", [KERNEL_VERSION, 1], F32, kind="ExternalInput")
    out = nc.dram_tensor("out", [bs, rnn], F32, kind="ExternalOutput")

    with tile.TileContext(nc) as tc:
        with (
            tc.tile_pool(name="singles", bufs=1) as singles,
            tc.tile_pool(name="ppool", bufs=bs) as ppool,
            tc.tile_pool(name="fpool", bufs=bs) as fpool,
            tc.tile_pool(name="sm", bufs=bs) as smpool,
            tc.tile_pool(name="respool", bufs=bs) as respool,
        ):
            # ---- constants (small, first on the DMA ring) ----
            hT_sb = singles.tile([128, kch, bs], BF16)
            nc.sync.dma_start(out=hT_sb, in_=hT[:, :, :])
            whT4_sb = singles.tile([128, kch, ACH, 128], FP8E4)
            nc.sync.dma_start(out=whT4_sb, in_=whT4[:, :, :, :])
            bhT_sb = singles.tile([128, ACH], F32)
            nc.sync.dma_start(out=bhT_sb, in_=bhT[:, :])
            waT_sb = singles.tile([128, ACH], BF16)
            nc.sync.dma_start(out=waT_sb, in_=waT[:, :])
            keep_sb = singles.tile([128, bs, nch], F32)
            nc.sync.dma_start(out=keep_sb, in_=keep[:, :, :])
            ver_sb = singles.tile([KERNEL_VERSION, 1], F32)
            nc.sync.dma_start(out=ver_sb, in_=ver[:, :])
            ident = singles.tile([1, 1], F32)
            nc.vector.memset(ident, 1.0)
            attbT = singles.tile([128, ACH, bs], F32)

            # ---- input DMAs, issued up front in wave order ----
            ptiles = [ppool.tile([128, ACH, lp], FP8E3, tag="p", name=f"p{b}")
                      for b in range(bs)]
            ttiles = [ppool.tile([128, ACH, lp], BF16, tag="t", name=f"t{b}")
                      for b in range(bs)]
            ftiles = [fpool.tile([128, nch, rnn], BF16, tag="f", name=f"f{b}")
                      for b in range(bs)]

            def dma_p(b):
                for c in range(0, ACH, 2):
                    nc.sync.dma_start(out=ptiles[b][:, c:c + 2, :],
                                      in_=p[b, :, c:c + 2, :])

            def dma_f(b):
                # 3-chunk groups so phase B can start on the early
                # chunks while the rest of f[b] is still on the wire
                for g in range(0, nch, 3):
                    sl = slice(g, min(g + 3, nch))
                    nc.sync.dma_start(out=ftiles[b][:, sl, :],
                                      in_=f[b, :, sl, :])

            dma_p(0)
            dma_p(1)
            dma_f(0)
            dma_p(2)
            dma_f(1)
            dma_p(3)
            dma_f(2)
            dma_f(3)

            # ---- phase 0: attbT[aq, c, b] = (h @ W_h.T + b_h).T ----
            with tc.tile_pool(name="ps0", bufs=1, space="PSUM") as ps0:
                for c in range(ACH):
                    at_ps = ps0.tile([128, bs], F32, tag=f"at{c % 2}",
                                     name=f"at{c}")
                    for k in range(kch):
                        nc.tensor.matmul(at_ps, lhsT=whT4_sb[:, k, c, :],
                                         rhs=hT_sb[:, k, :],
                                         start=(k == 0), stop=(k == kch - 1))
                    nc.vector.tensor_scalar_add(attbT[:, c, :], at_ps,
                                                bhT_sb[:, c:c + 1])

            # ---- main pipeline ----
            with tc.tile_pool(name="pssc", bufs=1, space="PSUM") as pssc, \
                 tc.tile_pool(name="pswt", bufs=1, space="PSUM") as pswt, \
                 tc.tile_pool(name="psacc", bufs=rps_bufs,
                              space="PSUM") as psacc:
                e_t = [smpool.tile([1, lp], F32, tag="e", name=f"e{b}")
                       for b in range(bs)]
                w_t = [smpool.tile([128, nch], BF16, tag="w", name=f"w{b}")
                       for b in range(bs)]
                z_t = [smpool.tile([128, 1], F32, tag="z", name=f"z{b}")
                       for b in range(bs)]
                zinv_t = [smpool.tile([1, 1], F32, tag="zi", name=f"zi{b}")
                          for b in range(bs)]

                def tanh(b):
                    for c in range(ACH):
                        nc.scalar.activation(out=ttiles[b][:, c, :],
                                             in_=ptiles[b][:, c, :],
                                             func=TANH,
                                             bias=attbT[:, c, b:b + 1])

                def scores(b):
                    sc = [pssc.tile([1, s.stop - s.start], F32, tag=f"sq{q}",
                                    name=f"sc{b}_{q}")
                          for q, s in enumerate(qsl)]
                    for q, s in enumerate(qsl):
                        for c in range(ACH):
                            nc.tensor.matmul(sc[q], lhsT=waT_sb[:, c:c + 1],
                                             rhs=ttiles[b][:, c, s],
                                             start=(c == 0),
                                             stop=(c == ACH - 1))
                    for q, s in enumerate(qsl):
                        nc.scalar.activation(out=e_t[b][:, s], in_=sc[q],
                                             func=EXP)

                def make_w(b):
                    wt_ps = pswt.tile([128, nch], F32, tag="wt",
                                      name=f"wt{b}")
                    for s in range(nch):
                        nc.tensor.transpose(
                            wt_ps[:, s:s + 1],
                            e_t[b][:, s * 128:(s + 1) * 128], ident)
                    nc.vector.tensor_mul(w_t[b], wt_ps, keep_sb[:, b, :])
                    zpart = smpool.tile([128, 1], F32, tag="zp",
                                        name=f"zp{b}")
                    nc.vector.reduce_sum(zpart, w_t[b],
                                         axis=mybir.AxisListType.X)
                    nc.gpsimd.partition_all_reduce(
                        z_t[b], zpart, channels=128,
                        reduce_op=bass_isa.ReduceOp.add)
                    nc.vector.reciprocal(zinv_t[b], z_t[b][0:1, :])

                def phase_b(b, rps):
                    for j in range(nch):
                        lhs = w_t[b][:, j:j + 1]
                        for hh in range(nh):
                            nc.tensor.matmul(
                                rps[hh], lhsT=lhs,
                                rhs=ftiles[b][:, j, hh * rh:(hh + 1) * rh],
                                start=(j == 0), stop=(j == nch - 1))

                def scale_out(b, rps):
                    res = respool.tile([1, rnn], F32, tag="res",
                                       name=f"res{b}")
                    for hh in range(nh):
                        nc.vector.tensor_scalar_mul(
                            res[:, hh * rh:(hh + 1) * rh], rps[hh],
                            zinv_t[b])
                    nc.sync.dma_start(out=out[b:b + 1, :], in_=res)

                pending = []
                for b in range(bs):
                    tanh(b)
                    scores(b)
                    make_w(b)
                    # free the PSUM slot this b will reuse first
                    while len(pending) >= rps_bufs:
                        bb, rr = pending.pop(0)
                        scale_out(bb, rr)
                    rps = [psacc.tile([1, rh], F32, tag=f"r{hh}",
                                      name=f"rps{b}_{hh}")
                           for hh in range(nh)]
                    phase_b(b, rps)
                    pending.append((b, rps))
                for bb, rr in pending:
                    scale_out(bb, rr)
    nc.finalize()
    return nc


_PROG = None
_PROG_NCH = None


def _get_program(nch):
    global _PROG, _PROG_NCH
    if _PROG is None or _PROG_NCH != nch:
        _PROG = build_program(nch=nch)
        _PROG_NCH = nch
    return _PROG


def make_in_maps(h, att_feats, p_att_feats, mask, W_h, b_h, W_a):
    h = np.asarray(h, dtype=np.float32)
    att_feats = np.asarray(att_feats, dtype=np.float32)
    p_att_feats = np.asarray(p_att_feats, dtype=np.float32)
    mask = np.asarray(mask)

    # l-compaction: keep only unmasked positions, pad to a common LP
    keep_idx = [np.nonzero(~mask[b])[0] for b in range(B)]
    nkeep = np.array([len(ix) for ix in keep_idx])
    LP = max(128, int(-(-nkeep.max() // 128)) * 128)
    nch = LP // 128

    bf16 = ml_dtypes.bfloat16
    fp8p = ml_dtypes.float8_e3m4
    fp8w = ml_dtypes.float8_e4m3
    kch = RNN // 128
    # rnn index = q*kch + c  <=>  plain reshape(128, kch)
    hT = np.ascontiguousarray(h.T.reshape(128, kch, B).astype(bf16))
    # whT4[kq, kc, c, aq] = W_h.T[kq*kch+kc, aq*ACH+c]
    whT4 = np.ascontiguousarray(
        np.asarray(W_h, np.float32).T.reshape(128, kch, 128, ACH)
        .transpose(0, 1, 3, 2).astype(fp8w))
    bhT = np.ascontiguousarray(
        np.asarray(b_h, np.float32).reshape(128, ACH))
    waT = np.ascontiguousarray(
        np.asarray(W_a, np.float32).reshape(128, ACH).astype(bf16))
    ver = np.zeros((KERNEL_VERSION, 1), np.float32)

    pc = np.zeros((B, 128, ACH, LP), dtype=fp8p)
    fc = np.zeros((B, 128, nch, RNN), dtype=bf16)
    keepm = np.zeros((B, 128, nch), dtype=np.float32)
    ar = np.arange(LP)
    for b in range(B):
        ix = keep_idx[b]
        nb = len(ix)
        # pT: [A, LP] with a = q*ACH + c
        pt = np.zeros((A, LP), dtype=fp8p)
        pt[:, :nb] = p_att_feats[b][ix].T.astype(fp8p)
        pc[b] = pt.reshape(128, ACH, LP)
        # f: l = n*128 + q (chunk-major, matches transpose output order)
        fr = np.zeros((LP, RNN), dtype=bf16)
        fr[:nb] = att_feats[b][ix].astype(bf16)
        fc[b] = fr.reshape(nch, 128, RNN).transpose(1, 0, 2)
        keepm[b] = (ar < nb).astype(np.float32).reshape(nch, 128).T

    in_maps = []
    for c in range(N_CORES):
        s = slice(c * BS, (c + 1) * BS)
        in_maps.append({
            "p": np.ascontiguousarray(pc[s]),
            "f": np.ascontiguousarray(fc[s]),
            "hT": np.ascontiguousarray(hT[:, :, s]),
            "whT4": whT4,
            "bhT": bhT,
            "waT": waT,
            "keep": np.ascontiguousarray(keepm[s].transpose(1, 0, 2)),
            "ver": ver,
        })
    return in_maps, nch


def run_sharded(inputs, trace=False, **kwargs):
    in_maps, nch = make_in_maps(
        inputs["h"], inputs["att_feats"], inputs["p_att_feats"],
        inputs["mask"], inputs["W_h"], inputs["b_h"], inputs["W_a"])
    nc = _get_program(nch)
    return run_bass_kernel_spmd(nc, in_maps, core_ids=list(range(N_CORES)),
                                trace=trace, **kwargs)


def kernel(h, att_feats, p_att_feats, mask, W_h, b_h, W_a, b_a):
    res = run_sharded({
        "h": h, "att_feats": att_feats, "p_att_feats": p_att_feats,
        "mask": mask, "W_h": W_h, "b_h": b_h, "W_a": W_a, "b_a": b_a})
    return np.concatenate([res.results[c]["out"] for c in range(N_CORES)],
                          axis=0).astype(np.float32)
